# revision 7
# baseline (speedup 1.0000x reference)
"""Deformable conv (AdaptiveConv) Trainium2 Bass kernel, 8-core data-parallel.

Strategy per core (each core owns half an image = 2048 output pixels):
  - x is host-relaid to a padded pixel-major bf16 image [4352 slots, 256 ch]
    (slot v = 66*y + x + 1, zero pad columns at x=-1/64) living in DRAM.
  - Bilinear sample positions / weights are computed on-device (DVE) from the
    offset tensor; corner-pair indices become int16 dma_gather indices.
  - dma_gather (non-transpose, elem_step=256) fetches pixel-PAIRS
    [x0,x1] x 256ch (1KB) per (pixel, tap, row-pair) -> G[(r,a) rows, 512 ch].
  - The 4-corner bilinear blend runs on TensorE as matmuls against per-group
    diagonal weight matrices D = mask * wv (built by one DVE tensor_scalar
    each), accumulating sampled features S[c, px] in PSUM.
  - The 3x3x256 conv is 18 accumulated matmuls per 256-px block with host
    pre-transposed bf16 weights; ReLU on ScalarE; f32 out.
"""
import numpy as np
import ml_dtypes

import concourse.bass as bass
import concourse.mybir as mybir
from concourse.tile import TileContext
from concourse import bass_utils
import concourse.bacc as bacc

F32 = mybir.dt.float32
BF16 = mybir.dt.bfloat16
I16 = mybir.dt.int16
I32 = mybir.dt.int32
OP = mybir.AluOpType
ACTF = mybir.ActivationFunctionType

# problem constants
N, C, H, W, CO, K2 = 4, 256, 64, 64, 256, 9
NCORES = 8
PXC = 2048          # output pixels per core (32 rows)
ROWSC = 32          # rows per core
NCALLS = 8          # gather calls per core; each covers 4 rows = 256 px
ROWSPC = ROWSC // NCALLS      # rows (64-px groups) per call = 4
NIDX = 256 * ROWSPC // 4 * 9 * 2 // 16 * 16  # = 4608 idxs per call
XSLOTS = 4352       # padded image slots (>= 66*64 + 2 = 4226)
VMAX = 4223         # max gatherable slot index (slot v covers v, v+1)

_CACHE = {}
DBG_CALLS = NCALLS  # debug knob: build only the first n 256-px calls
DBG_GATHER = 'full'  # 'full' | 'skip' | '2'/'4'/... (split count)


def _build_program():
    nc = bacc.Bacc('TRN2', num_devices=NCORES)

    d_xp = nc.dram_tensor('xp', [XSLOTS * C], BF16, kind='ExternalInput')
    d_wt = nc.dram_tensor('wt', [128, K2 * 2 * 2 * 128], BF16, kind='ExternalInput')
    d_offAy = nc.dram_tensor('offAy', [128, 288], F32, kind='ExternalInput')
    d_offAx = nc.dram_tensor('offAx', [128, 288], F32, kind='ExternalInput')
    d_bAy = nc.dram_tensor('bAy', [128, 288], F32, kind='ExternalInput')
    d_bAx = nc.dram_tensor('bAx', [128, 288], F32, kind='ExternalInput')
    d_offBy = nc.dram_tensor('offBy', [128, 288], F32, kind='ExternalInput')
    d_offBx = nc.dram_tensor('offBx', [128, 288], F32, kind='ExternalInput')
    d_bBy = nc.dram_tensor('bBy', [128, 288], F32, kind='ExternalInput')
    d_bBx = nc.dram_tensor('bBx', [128, 288], F32, kind='ExternalInput')
    d_cB = nc.dram_tensor('cB', [128, 288], F32, kind='ExternalInput')
    d_mask = nc.dram_tensor('maskA', [128, 64], BF16, kind='ExternalInput')
    d_sA = nc.dram_tensor('sA', [128, 1], F32, kind='ExternalInput')
    d_tA = nc.dram_tensor('tA', [128, 1], F32, kind='ExternalInput')
    d_rA = nc.dram_tensor('rA', [128, 1], F32, kind='ExternalInput')
    d_out = nc.dram_tensor('out', [CO, PXC], F32, kind='ExternalOutput')

    gather_src = bass.AP(d_xp, 0, [[C, VMAX + 1], [1, 2 * C]])

    with TileContext(nc) as tc:
        with tc.tile_pool(name='const', bufs=1) as cpool, \
             tc.tile_pool(name='pipe', bufs=1) as ppool, \
             tc.tile_pool(name='gp', bufs=2) as gpool, \
             tc.tile_pool(name='sp', bufs=2) as spool, \
             tc.tile_pool(name='dp', bufs=8) as dpool, \
             tc.tile_pool(name='op', bufs=2) as opool, \
             tc.tile_pool(name='pb', bufs=3, space='PSUM') as pbpool, \
             tc.tile_pool(name='po', bufs=2, space='PSUM') as popool:

            def load(dram, shape, dtype, pool=cpool, tag=None):
                t = pool.tile(shape, dtype, tag=tag or dram.name + '_t')
                nc.sync.dma_start(t[:], dram.ap())
                return t

            t_wt = load(d_wt, [128, K2 * 2 * 2 * 128], BF16)
            t_mask = load(d_mask, [128, 64], BF16)
            t_sA = load(d_sA, [128, 1], F32)
            t_tA = load(d_tA, [128, 1], F32)
            t_rA = load(d_rA, [128, 1], F32)
            t_offAy = load(d_offAy, [128, 288], F32)
            t_offAx = load(d_offAx, [128, 288], F32)
            t_bAy = load(d_bAy, [128, 288], F32)
            t_bAx = load(d_bAx, [128, 288], F32)
            t_offBy = load(d_offBy, [128, 288], F32)
            t_offBx = load(d_offBx, [128, 288], F32)
            t_bBy = load(d_bBy, [128, 288], F32)
            t_bBx = load(d_bBx, [128, 288], F32)
            t_cB = load(d_cB, [128, 288], F32)

            def floor16(z, tag):
                """floor of shifted-positive z (exact, cast-mode independent)."""
                ti = ppool.tile([128, 288], I32, tag=f'{tag}_i')
                nc.vector.tensor_copy(ti[:], z[:])
                tf = ppool.tile([128, 288], F32, tag=f'{tag}_f')
                nc.vector.tensor_copy(tf[:], ti[:])
                ov = ppool.tile([128, 288], F32, tag=f'{tag}_ov')
                nc.vector.tensor_tensor(ov[:], tf[:], z[:], OP.is_gt)
                nc.vector.tensor_tensor(tf[:], tf[:], ov[:], OP.subtract)
                return tf

            # ---- layout-A pipeline: bilinear weights wvx0/wvx1 [128,288] ----
            pyP = ppool.tile([128, 288], F32, tag='pyP')
            nc.vector.tensor_tensor(pyP[:], t_offAy[:], t_bAy[:], OP.add)
            y0P = floor16(pyP, 'y0A')
            fy = ppool.tile([128, 288], F32, tag='fy')
            nc.vector.tensor_tensor(fy[:], pyP[:], y0P[:], OP.subtract)
            wy = ppool.tile([128, 288], F32, tag='wy')
            nc.vector.tensor_scalar(wy[:], fy[:], t_sA[:], t_tA[:], OP.mult, OP.add)
            yvP = ppool.tile([128, 288], F32, tag='yvP')
            nc.vector.tensor_scalar(yvP[:], y0P[:], t_rA[:], None, OP.add)
            m1 = ppool.tile([128, 288], F32, tag='m1')
            nc.vector.tensor_scalar(m1[:], yvP[:], 16.0, None, OP.is_ge)
            m2 = ppool.tile([128, 288], F32, tag='m2')
            nc.vector.tensor_scalar(m2[:], yvP[:], 79.0, None, OP.is_le)
            nc.vector.tensor_tensor(m1[:], m1[:], m2[:], OP.mult)
            nc.vector.tensor_tensor(wy[:], wy[:], m1[:], OP.mult)   # wy_eff

            pxP = ppool.tile([128, 288], F32, tag='pxP')
            nc.vector.tensor_tensor(pxP[:], t_offAx[:], t_bAx[:], OP.add)
            x0P = floor16(pxP, 'x0A')
            fx = ppool.tile([128, 288], F32, tag='fx')
            nc.vector.tensor_tensor(fx[:], pxP[:], x0P[:], OP.subtract)
            # x0 corner: weight (1-fx) * (0 <= x0 <= 63)
            wx = ppool.tile([128, 288], F32, tag='wx')
            nc.vector.tensor_scalar(wx[:], fx[:], -1.0, 1.0, OP.mult, OP.add)
            nc.vector.tensor_scalar(m1[:], x0P[:], 16.0, None, OP.is_ge)
            nc.vector.tensor_scalar(m2[:], x0P[:], 79.0, None, OP.is_le)
            nc.vector.tensor_tensor(m1[:], m1[:], m2[:], OP.mult)
            nc.vector.tensor_tensor(wx[:], wx[:], m1[:], OP.mult)
            wvx0 = cpool.tile([128, 288], F32, tag='wvx0')
            nc.vector.tensor_tensor(wvx0[:], wy[:], wx[:], OP.mult)
            # x1 corner: weight fx * (-1 <= x0 <= 62)
            nc.vector.tensor_scalar(m1[:], x0P[:], 15.0, None, OP.is_ge)
            nc.vector.tensor_scalar(m2[:], x0P[:], 78.0, None, OP.is_le)
            nc.vector.tensor_tensor(m1[:], m1[:], m2[:], OP.mult)
            nc.vector.tensor_tensor(m1[:], fx[:], m1[:], OP.mult)
            wvx1 = cpool.tile([128, 288], F32, tag='wvx1')
            nc.vector.tensor_tensor(wvx1[:], wy[:], m1[:], OP.mult)

            # ---- layout-B pipeline: gather indices ----
            pyB = ppool.tile([128, 288], F32, tag='pyB')
            nc.vector.tensor_tensor(pyB[:], t_offBy[:], t_bBy[:], OP.add)
            y0B = floor16(pyB, 'y0B')
            pxB = ppool.tile([128, 288], F32, tag='pxB')
            nc.vector.tensor_tensor(pxB[:], t_offBx[:], t_bBx[:], OP.add)
            x0B = floor16(pxB, 'x0B')
            v = ppool.tile([128, 288], F32, tag='v')
            nc.vector.tensor_scalar(v[:], y0B[:], 66.0, None, OP.mult)
            nc.vector.tensor_tensor(v[:], v[:], x0B[:], OP.add)
            nc.vector.tensor_tensor(v[:], v[:], t_cB[:], OP.add)
            nc.vector.tensor_scalar(v[:], v[:], 0.0, float(VMAX), OP.max, OP.min)
            xb = ppool.tile([128, 288], I16, tag='xb16')
            nc.vector.tensor_copy(xb[:], v[:])

            # fold [128,288] -> idx tile Y [128, 8cb*9k*8u*4ahi = 2304]
            t_Y = cpool.tile([128, NCALLS * 288], I16, tag='Y')
            xbv = xb[:].rearrange('p (cb k ahi) -> p cb k ahi', cb=8, k=9)
            yv4 = t_Y[:].rearrange('p (cb k u ahi) -> p cb k u ahi', cb=8, k=9, u=8)
            for u in range(8):
                nc.sync.dma_start(yv4[0:16, :, :, u, :], xbv[u * 16:u * 16 + 16])
            for w_ in range(1, 8):
                nc.sync.dma_start(t_Y[16 * w_:16 * w_ + 16, :], t_Y[0:16, :])

            # ---- main loop over 256-px calls ----
            for cb in range(DBG_CALLS):
                t_G = gpool.tile([128, 36, 512], BF16, tag='G')
                # one 512-idx gather per tap (descriptor-ring carveout caps
                # a single dma_gather somewhere between 768 and 1152 idxs)
                for k in range(K2):
                    nc.gpsimd.dma_gather(
                        t_G[:, k * 4:(k + 1) * 4, :], gather_src,
                        t_Y[:, cb * 288 + k * 32: cb * 288 + (k + 1) * 32],
                        512, 512, 2 * C, elem_step=C)

                t_S = spool.tile([128, K2, 2, 256], BF16, tag='S')
                for k in range(K2):
                    pb = pbpool.tile([128, 512], F32, tag='pb')
                    for gl in range(ROWSPC):
                        col = (ROWSPC * cb + gl) * K2 + k
                        D0 = dpool.tile([128, 64], BF16, tag='D0')
                        nc.vector.tensor_scalar(
                            D0[:], t_mask[:], wvx0[:, col:col + 1], None, OP.mult)
                        D1 = dpool.tile([128, 64], BF16, tag='D1')
                        nc.vector.tensor_scalar(
                            D1[:], t_mask[:], wvx1[:, col:col + 1], None, OP.mult)
                        j = k * ROWSPC + gl
                        for ct in range(2):
                            sl = slice(ct * 256 + gl * 64, ct * 256 + gl * 64 + 64)
                            nc.tensor.matmul(
                                pb[:, sl], t_G[:, j, ct * 128:ct * 128 + 128],
                                D0[:], start=True, stop=False)
                            nc.tensor.matmul(
                                pb[:, sl], t_G[:, j, 256 + ct * 128:256 + ct * 128 + 128],
                                D1[:], start=False, stop=True)
                    for ct in range(2):
                        nc.scalar.activation(
                            t_S[:, k, ct, :], pb[:, ct * 256:(ct + 1) * 256], ACTF.Copy)

                po = popool.tile([128, 512], F32, tag='po')
                for ot in range(2):
                    for ki in range(18):
                        k, ct = divmod(ki, 2)
                        wcol = (k * 2 + ct) * 2 + ot
                        nc.tensor.matmul(
                            po[:, ot * 256:(ot + 1) * 256],
                            t_wt[:, wcol * 128:(wcol + 1) * 128],
                            t_S[:, k, ct, :],
                            start=(ki == 0), stop=(ki == 17))
                ro = opool.tile([128, 2, 256], F32, tag='ro')
                for ot in range(2):
                    nc.scalar.activation(
                        ro[:, ot, :], po[:, ot * 256:(ot + 1) * 256], ACTF.Relu)
                    nc.sync.dma_start(
                        d_out.ap()[ot * 128:(ot + 1) * 128,
                                   cb * 256:(cb + 1) * 256],
                        ro[:, ot, :])

    nc.compile()
    return nc


def _prep_inputs(x, offset, weight):
    """Host-side shard/relayout: per-core input dicts."""
    x = np.asarray(x, np.float32)
    offset = np.asarray(offset, np.float32)
    weight = np.asarray(weight, np.float32)

    # padded pixel-major bf16 images
    xps = []
    slot = (66 * np.arange(H)[:, None] + np.arange(W)[None, :] + 1).ravel()
    for n in range(N):
        xp = np.zeros((XSLOTS, C), ml_dtypes.bfloat16)
        xp[slot] = x[n].transpose(1, 2, 0).reshape(H * W, C)
        xps.append(xp.reshape(-1))

    # weights: wt[c_lo, (k, ct, ot, o_lo)]
    wr = weight.reshape(2, 128, 2, 128, K2)       # [ot, o_lo, ct, c_lo, k]
    wt_host = np.ascontiguousarray(
        wr.transpose(3, 4, 2, 0, 1).reshape(128, K2 * 2 * 2 * 128)
    ).astype(ml_dtypes.bfloat16)

    # static per-partition / mask constants
    p = np.arange(128)
    maskA = np.zeros((128, 64), ml_dtypes.bfloat16)
    maskA[p, p % 64] = 1.0
    sA = np.where(p < 64, -1.0, 1.0).astype(np.float32).reshape(128, 1)
    tA = np.where(p < 64, 1.0, 0.0).astype(np.float32).reshape(128, 1)
    rA = (p // 64).astype(np.float32).reshape(128, 1)

    # layout-A index grids
    aA = p % 64
    cA = np.arange(288)
    gA, kA = cA // K2, cA % K2
    kyA, kxA = kA // 3 - 1, kA % 3 - 1
    pxA = gA[None, :] * 64 + aA[:, None]          # [128,288] pixel-in-core
    bAx = (aA[:, None] + kxA[None, :] + 16.0).astype(np.float32) * np.ones((128, 288), np.float32)

    # layout-B index grids
    uB, qB = p // 16, p % 16
    glB, rB = uB // 2, uB % 2
    CB = np.arange(288)
    cbB, kB, ahiB = CB // 36, (CB % 36) // 4, CB % 4
    kyB, kxB = kB // 3 - 1, kB % 3 - 1
    wB = ahiB[None, :] * 16 + qB[:, None]
    rowB = cbB[None, :] * ROWSPC + glB[:, None]
    pxB = rowB * 64 + wB
    bBx = (wB + kxB[None, :] + 16.0).astype(np.float32)
    cBv = (66.0 * rB[:, None] - 1071.0) * np.ones((128, 288), np.float32)

    in_maps = []
    for core in range(NCORES):
        img, half = core // 2, core % 2
        h0 = half * ROWSC
        offs = offset[img * H * W + h0 * W: img * H * W + h0 * W + PXC]
        bAy = ((h0 + gA)[None, :] + kyA[None, :] + 16.0).astype(np.float32) \
            * np.ones((128, 288), np.float32)
        bBy = ((h0 + rowB) + kyB[None, :] + 16.0).astype(np.float32)
        in_maps.append({
            'xp': xps[img],
            'wt': wt_host,
            'offAy': np.ascontiguousarray(offs[pxA, 2 * kA[None, :]]),
            'offAx': np.ascontiguousarray(offs[pxA, 2 * kA[None, :] + 1]),
            'bAy': bAy, 'bAx': np.ascontiguousarray(bAx.astype(np.float32)),
            'offBy': np.ascontiguousarray(offs[pxB, 2 * kB[None, :]]),
            'offBx': np.ascontiguousarray(offs[pxB, 2 * kB[None, :] + 1]),
            'bBy': np.ascontiguousarray(bBy), 'bBx': np.ascontiguousarray(bBx),
            'cB': np.ascontiguousarray(cBv.astype(np.float32)),
            'maskA': maskA, 'sA': sA, 'tA': tA, 'rA': rA,
        })
    return in_maps


def kernel(x, offset, weight, _run_kwargs=None):
    if 'nc' not in _CACHE:
        _CACHE['nc'] = _build_program()
    nc = _CACHE['nc']
    in_maps = _prep_inputs(x, offset, weight)
    res = bass_utils.run_bass_kernel_spmd(
        nc, in_maps, core_ids=list(range(NCORES)), **(_run_kwargs or {}))
    out = np.empty((N, CO, H, W), np.float32)
    for core in range(NCORES):
        img, half = core // 2, core % 2
        out[img, :, half * ROWSC:(half + 1) * ROWSC, :] = \
            res.results[core]['out'].reshape(CO, ROWSC, W)
    _CACHE['last_result'] = res
    return out


# revision 9
# speedup vs baseline: 1.5542x; 1.5542x over previous
"""Deformable conv (AdaptiveConv) Trainium2 Bass kernel, 8-core data-parallel.

Strategy per core (each core owns half an image = 2048 output pixels):
  - x is host-relaid to a row-pair-interleaved, x-padded pixel-major bf16
    image: record s = (66*y + x + 1)*2 + r holds channels of pixel (y+r, x).
    One 2KB dma_gather descriptor at v = (66*y0 + x0 + 1)*2 then fetches all
    FOUR bilinear corners (y0/y1 x x0/x1) of one (pixel, tap) sample — this
    halves the GPSIMD descriptor-generation work, which profiling showed to
    be the bottleneck (~7ns of Q7 time per gathered index).
  - Sample positions / bilinear weights are computed on-device (DVE) from the
    offset tensor; corner indices become int16 dma_gather indices via a
    16-partition fold + replication.
  - The 4-corner blend runs on TensorE: matmuls against per-128-px-group
    diagonal weight matrices D_j = I * wv_j (one tensor_scalar each, built on
    DVE/ACT), accumulating sampled features S[c, px] in PSUM.
  - The 3x3x256 conv is 18 accumulated matmuls per 256-px block with host
    pre-transposed bf16 weights; ReLU on ScalarE; f32 out.
"""
import numpy as np
import ml_dtypes

import concourse.bass as bass
import concourse.mybir as mybir
from concourse.tile import TileContext
from concourse import bass_utils
import concourse.bacc as bacc

F32 = mybir.dt.float32
BF16 = mybir.dt.bfloat16
I16 = mybir.dt.int16
I32 = mybir.dt.int32
OP = mybir.AluOpType
ACTF = mybir.ActivationFunctionType

# problem constants
N, C, H, W, CO, K2 = 4, 256, 64, 64, 256, 9
NCORES = 8
PXC = 2048          # output pixels per core (32 rows)
ROWSC = 32          # rows per core
NCALLS = 8          # 256-px blocks per core
XREC = 8704         # records in the interleaved padded image (rows y=-1..63)
VMAX2 = 8578        # max gatherable record index (elem covers v..v+3)
GROWS = VMAX2 + 1   # gather-source row count

_CACHE = {}
DBG_CALLS = NCALLS


def _build_program():
    nc = bacc.Bacc('TRN2', num_devices=NCORES)

    d_xq = nc.dram_tensor('xq', [XREC * C], BF16, kind='ExternalInput')
    d_wt = nc.dram_tensor('wt', [128, K2 * 2 * 2 * 128], BF16, kind='ExternalInput')
    d_offAy = nc.dram_tensor('offAy', [128, 144], F32, kind='ExternalInput')
    d_offAx = nc.dram_tensor('offAx', [128, 144], F32, kind='ExternalInput')
    d_bAy = nc.dram_tensor('bAy', [128, 144], F32, kind='ExternalInput')
    d_bAx = nc.dram_tensor('bAx', [128, 144], F32, kind='ExternalInput')
    d_offBy = nc.dram_tensor('offBy', [128, 192], F32, kind='ExternalInput')
    d_offBx = nc.dram_tensor('offBx', [128, 192], F32, kind='ExternalInput')
    d_bBy = nc.dram_tensor('bBy', [128, 192], F32, kind='ExternalInput')
    d_bBx = nc.dram_tensor('bBx', [128, 192], F32, kind='ExternalInput')
    d_mask = nc.dram_tensor('maskI', [128, 128], BF16, kind='ExternalInput')
    d_out = nc.dram_tensor('out', [CO, PXC], F32, kind='ExternalOutput')

    gather_src = bass.AP(d_xq, 0, [[C, GROWS], [1, 4 * C]])

    with TileContext(nc) as tc:
        with tc.tile_pool(name='const', bufs=1) as cpool, \
             tc.tile_pool(name='pipe', bufs=1) as ppool, \
             tc.tile_pool(name='gp', bufs=2) as gpool, \
             tc.tile_pool(name='sp', bufs=2) as spool, \
             tc.tile_pool(name='dp', bufs=8) as dpool, \
             tc.tile_pool(name='op', bufs=2) as opool, \
             tc.tile_pool(name='pb', bufs=3, space='PSUM') as pbpool, \
             tc.tile_pool(name='po', bufs=2, space='PSUM') as popool:

            def load(dram, shape, dtype, pool=cpool):
                t = pool.tile(shape, dtype, tag=dram.name + '_t')
                nc.sync.dma_start(t[:], dram.ap())
                return t

            t_wt = load(d_wt, [128, K2 * 2 * 2 * 128], BF16)
            t_mask = load(d_mask, [128, 128], BF16)
            t_offAy = load(d_offAy, [128, 144], F32)
            t_offAx = load(d_offAx, [128, 144], F32)
            t_bAy = load(d_bAy, [128, 144], F32)
            t_bAx = load(d_bAx, [128, 144], F32)
            t_offBy = load(d_offBy, [128, 192], F32)
            t_offBx = load(d_offBx, [128, 192], F32)
            t_bBy = load(d_bBy, [128, 192], F32)
            t_bBx = load(d_bBx, [128, 192], F32)

            def floorp(z, cols, tag):
                """floor of shifted-positive z (exact, cast-mode independent)."""
                ti = ppool.tile([128, cols], I32, tag=f'{tag}_i')
                nc.vector.tensor_copy(ti[:], z[:])
                tf = ppool.tile([128, cols], F32, tag=f'{tag}_f')
                nc.vector.tensor_copy(tf[:], ti[:])
                ov = ppool.tile([128, cols], F32, tag=f'{tag}_ov')
                nc.vector.tensor_tensor(ov[:], tf[:], z[:], OP.is_gt)
                nc.vector.tensor_tensor(tf[:], tf[:], ov[:], OP.subtract)
                return tf

            # ---- layout-A pipeline: corner weights wv0..wv3 [128,144] ----
            # partition p = px % 128; col = jg*9 + k (jg = 128-px group)
            def axis_weights(toff, tbase, lo0, hi0, lo1, hi1, tag):
                pP = ppool.tile([128, 144], F32, tag=f'p{tag}')
                nc.vector.tensor_tensor(pP[:], toff[:], tbase[:], OP.add)
                f0 = floorp(pP, 144, f'f{tag}')
                fr = ppool.tile([128, 144], F32, tag=f'fr{tag}')
                nc.vector.tensor_tensor(fr[:], pP[:], f0[:], OP.subtract)
                w0 = ppool.tile([128, 144], F32, tag=f'w0{tag}')
                nc.vector.tensor_scalar(w0[:], fr[:], -1.0, 1.0, OP.mult, OP.add)
                m1 = ppool.tile([128, 144], F32, tag=f'm1{tag}')
                m2 = ppool.tile([128, 144], F32, tag=f'm2{tag}')
                nc.vector.tensor_scalar(m1[:], f0[:], lo0, None, OP.is_ge)
                nc.vector.tensor_scalar(m2[:], f0[:], hi0, None, OP.is_le)
                nc.vector.tensor_tensor(m1[:], m1[:], m2[:], OP.mult)
                nc.vector.tensor_tensor(w0[:], w0[:], m1[:], OP.mult)
                w1 = ppool.tile([128, 144], F32, tag=f'w1{tag}')
                nc.vector.tensor_scalar(m1[:], f0[:], lo1, None, OP.is_ge)
                nc.vector.tensor_scalar(m2[:], f0[:], hi1, None, OP.is_le)
                nc.vector.tensor_tensor(m1[:], m1[:], m2[:], OP.mult)
                nc.vector.tensor_tensor(w1[:], fr[:], m1[:], OP.mult)
                return w0, w1

            wy0, wy1 = axis_weights(t_offAy, t_bAy, 16.0, 79.0, 15.0, 78.0, 'ya')
            wx0, wx1 = axis_weights(t_offAx, t_bAx, 16.0, 79.0, 15.0, 78.0, 'xa')
            # corner order in the gathered elem: (y0x0),(y1x0),(y0x1),(y1x1)
            wv = []
            for j, (wy, wx) in enumerate(
                    [(wy0, wx0), (wy1, wx0), (wy0, wx1), (wy1, wx1)]):
                t = cpool.tile([128, 144], F32, tag=f'wv{j}')
                nc.vector.tensor_tensor(t[:], wy[:], wx[:], OP.mult)
                wv.append(t)

            # ---- layout-B pipeline: gather indices [128,192] ----
            # P = u*16 + q, u = kl*2 + jh (<6); C = cb*24 + kt*8 + pl
            pyB = ppool.tile([128, 192], F32, tag='pyB')
            nc.vector.tensor_tensor(pyB[:], t_offBy[:], t_bBy[:], OP.add)
            y0B = floorp(pyB, 192, 'y0B')
            pxB = ppool.tile([128, 192], F32, tag='pxB')
            nc.vector.tensor_tensor(pxB[:], t_offBx[:], t_bBx[:], OP.add)
            x0B = floorp(pxB, 192, 'x0B')
            # v = (66*(y0+1) + x0 + 1)*2 = 132*y0P + 2*x0P - 2010
            v = ppool.tile([128, 192], F32, tag='v')
            nc.vector.tensor_scalar(v[:], y0B[:], 132.0, None, OP.mult)
            v2 = ppool.tile([128, 192], F32, tag='v2')
            nc.vector.tensor_scalar(v2[:], x0B[:], 2.0, -2010.0, OP.mult, OP.add)
            nc.vector.tensor_tensor(v[:], v[:], v2[:], OP.add)
            nc.vector.tensor_scalar(v[:], v[:], 0.0, float(VMAX2), OP.max, OP.min)
            xb = ppool.tile([128, 192], I16, tag='xb16')
            nc.vector.tensor_copy(xb[:], v[:])

            # fold to Y [128, 8cb * 3kt * 48] int16; call (cb,kt): cols 48
            # within call: s = kl*16 + jh*8 + pl
            t_Y = cpool.tile([128, NCALLS * 3 * 48], I16, tag='Y')
            xbv = xb[:].rearrange('p (cb kt pl) -> p cb kt pl', cb=8, kt=3)
            yv4 = t_Y[:].rearrange('p (cb kt pl) -> p cb kt pl', cb=8, kt=3, pl=48)
            for u in range(6):
                kl, jh = u // 2, u % 2
                base = kl * 16 + jh * 8
                nc.sync.dma_start(
                    yv4[0:16, :, :, base:base + 8], xbv[u * 16:u * 16 + 16])
            for w_ in range(1, 8):
                nc.sync.dma_start(t_Y[16 * w_:16 * w_ + 16, :], t_Y[0:16, :])

            # ---- main loop over 256-px blocks ----
            for cb in range(DBG_CALLS):
                t_G = gpool.tile([128, 18, 1024], BF16, tag='G')
                for kt in range(3):
                    # 768 idxs: taps 3kt..3kt+2; G block j = k*2 + jh
                    nc.gpsimd.dma_gather(
                        t_G[:, kt * 6:(kt + 1) * 6, :], gather_src,
                        t_Y[:, (cb * 3 + kt) * 48:(cb * 3 + kt + 1) * 48],
                        768, 768, 4 * C, elem_step=C)

                t_S = spool.tile([128, K2, 2, 256], BF16, tag='S')
                for k in range(K2):
                    pb = pbpool.tile([128, 512], F32, tag='pb')
                    for jh in range(2):
                        col = (cb * 2 + jh) * K2 + k
                        Ds = []
                        for j in range(4):
                            D = dpool.tile([128, 128], BF16, tag=f'D{j}')
                            if j in (0, 1):
                                nc.vector.tensor_scalar(
                                    D[:], t_mask[:], wv[j][:, col:col + 1],
                                    None, OP.mult)
                            else:
                                nc.scalar.activation(
                                    D[:], t_mask[:], ACTF.Copy,
                                    scale=wv[j][:, col:col + 1])
                            Ds.append(D)
                        for ct in range(2):
                            sl = slice(ct * 256 + jh * 128, ct * 256 + jh * 128 + 128)
                            for j in range(4):
                                nc.tensor.matmul(
                                    pb[:, sl],
                                    t_G[:, k * 2 + jh, j * 256 + ct * 128:
                                        j * 256 + ct * 128 + 128],
                                    Ds[j][:], start=(j == 0), stop=(j == 3))
                    # cast PSUM f32 -> SBUF bf16 (alternate DVE/ACT)
                    pbv = pb[:].rearrange('p (a b) -> p a b', a=2)
                    if k % 2 == 0:
                        nc.vector.tensor_copy(t_S[:, k, :, :], pbv)
                    else:
                        nc.scalar.activation(t_S[:, k, :, :], pbv, ACTF.Copy)

                po = popool.tile([128, 512], F32, tag='po')
                for ot in range(2):
                    for ki in range(18):
                        k, ct = divmod(ki, 2)
                        wcol = (k * 2 + ct) * 2 + ot
                        nc.tensor.matmul(
                            po[:, ot * 256:(ot + 1) * 256],
                            t_wt[:, wcol * 128:(wcol + 1) * 128],
                            t_S[:, k, ct, :],
                            start=(ki == 0), stop=(ki == 17))
                ro = opool.tile([128, 2, 256], F32, tag='ro')
                for ot in range(2):
                    nc.scalar.activation(
                        ro[:, ot, :], po[:, ot * 256:(ot + 1) * 256], ACTF.Relu)
                    nc.sync.dma_start(
                        d_out.ap()[ot * 128:(ot + 1) * 128,
                                   cb * 256:(cb + 1) * 256],
                        ro[:, ot, :])

    nc.compile()
    return nc


def _prep_inputs(x, offset, weight):
    """Host-side shard/relayout: per-core input dicts."""
    x = np.asarray(x, np.float32)
    offset = np.asarray(offset, np.float32)
    weight = np.asarray(weight, np.float32)

    # row-pair interleaved, x-padded pixel-major bf16 images
    xqs = []
    yy = np.arange(H)[:, None]
    xx = np.arange(W)[None, :]
    s_even = ((66 * (yy + 1) + xx + 1) * 2).ravel()
    for n in range(N):
        pix = x[n].transpose(1, 2, 0).reshape(H * W, C).astype(ml_dtypes.bfloat16)
        xq = np.zeros((XREC, C), ml_dtypes.bfloat16)
        xq[s_even] = pix                    # record (y, x, r=0) = pixel (y, x)
        xq[s_even - 131] = pix              # record (y-1, x, r=1) = pixel (y, x)
        xqs.append(xq.reshape(-1))

    # weights: wt[c_lo, (k, ct, ot, o_lo)]
    wr = weight.reshape(2, 128, 2, 128, K2)       # [ot, o_lo, ct, c_lo, k]
    wt_host = np.ascontiguousarray(
        wr.transpose(3, 4, 2, 0, 1).reshape(128, K2 * 2 * 2 * 128)
    ).astype(ml_dtypes.bfloat16)

    p = np.arange(128)
    maskI = np.zeros((128, 128), ml_dtypes.bfloat16)
    maskI[p, p] = 1.0

    # layout-A grids: p = px%128, col = jg*9 + k (jg = px//128, 16 per core)
    cA = np.arange(144)
    jgA, kA = cA // K2, cA % K2
    kyA, kxA = kA // 3 - 1, kA % 3 - 1
    pxA = jgA[None, :] * 128 + p[:, None]
    wA = pxA % W
    rowA = pxA // W
    bAx = (wA + kxA[None, :] + 16.0).astype(np.float32)

    # layout-B grids: P = (kl*2+jh)*16 + q; C = cb*24 + kt*8 + pl
    uB, qB = p // 16, p % 16
    klB, jhB = np.minimum(uB // 2, 2), uB % 2     # clamp dead rows (u>=6)
    CB = np.arange(192)
    cbB, ktB, plB = CB // 24, (CB % 24) // 8, CB % 8
    kB = ktB * 3 + klB[:, None]
    pxB = cbB[None, :] * 256 + jhB[:, None] * 128 + plB[None, :] * 16 + qB[:, None]
    kyB, kxB = kB // 3 - 1, kB % 3 - 1
    wB = pxB % W
    rowB = pxB // W
    bBx = (wB + kxB + 16.0).astype(np.float32)

    in_maps = []
    for core in range(NCORES):
        img, half = core // 2, core % 2
        h0 = half * ROWSC
        offs = offset[img * H * W + h0 * W: img * H * W + h0 * W + PXC]
        bAy = ((h0 + rowA) + kyA[None, :] + 16.0).astype(np.float32)
        bBy = ((h0 + rowB) + kyB + 16.0).astype(np.float32)
        in_maps.append({
            'xq': xqs[img],
            'wt': wt_host,
            'offAy': np.ascontiguousarray(offs[pxA, 2 * kA[None, :]]),
            'offAx': np.ascontiguousarray(offs[pxA, 2 * kA[None, :] + 1]),
            'bAy': np.ascontiguousarray(bAy),
            'bAx': np.ascontiguousarray(bAx),
            'offBy': np.ascontiguousarray(offs[pxB, 2 * kB]),
            'offBx': np.ascontiguousarray(offs[pxB, 2 * kB + 1]),
            'bBy': np.ascontiguousarray(bBy),
            'bBx': np.ascontiguousarray(bBx),
            'maskI': maskI,
        })
    return in_maps


def kernel(x, offset, weight, _run_kwargs=None):
    if 'nc' not in _CACHE:
        _CACHE['nc'] = _build_program()
    nc = _CACHE['nc']
    in_maps = _prep_inputs(x, offset, weight)
    res = bass_utils.run_bass_kernel_spmd(
        nc, in_maps, core_ids=list(range(NCORES)), **(_run_kwargs or {}))
    out = np.empty((N, CO, H, W), np.float32)
    for core in range(NCORES):
        img, half = core // 2, core % 2
        out[img, :, half * ROWSC:(half + 1) * ROWSC, :] = \
            res.results[core]['out'].reshape(CO, ROWSC, W)
    _CACHE['last_result'] = res
    return out
